# revision 1
# baseline (speedup 1.0000x reference)
"""Trainium2 Bass kernel for nn_Attention_22299470201527.

Dense transformer attention block:
  LayerNorm -> Wq/Wkv projections -> per-head QK RMSNorm -> 2D RoPE ->
  softmax(QK^T) V -> Wo projection,  B=8, N=1024, DIM=1024, H=16, DH=64.

Sharding: data-parallel over batch — 8 batch elements on 8 NeuronCores,
one per core, weights replicated, no collectives. kernel(**inputs) takes
the full unsharded inputs and returns the full [8, 1024, 1024] output.

Per-core device program:
  A) LayerNorm (token-major, bn_stats) then PE-transpose to feature-major
     xnT (float32r — full-rate fp32 matmul format).
  B) q/k/v projections token-major (stationary = xnT blocks, moving = W
     streamed from DRAM); per-head RMSNorm + 2D RoPE in token-major
     (free-dim ops only); q/k PE-transposed to feature-major fp16 qT/kT.
     v evicted straight into token-major bf16 V with an extra all-ones
     column per head (augmented V) for softmax denominators.
  C) Attention per head, software-pipelined so QK/exp of head h+1 issues
     before AV of head h (keeps the PE dense and the HAM clock warm):
     dots^T [k, q] = matmul(lhsT=kT, rhs=qT) in fp16; exp on ScalarE
     straight out of PSUM with NO max-subtraction (|dots| <= 64 because
     q/k are RMS-normalized to norm 8, and exp(64) fits fp32); then
     out_aug^T [65, q] = V_aug^T @ E (bf16) accumulated over k tiles.
     Row 64 is the softmax denominator. Per completed head pair the
     normalization (fast reciprocal + partition-broadcast via a DRAM
     bounce + multiply) streams in the shadow of later heads, writing
     normalized output into qT's storage (dead by then).
  D) Wo projection (fp16 weights) and DMA out token-major.

Host side folds gamma_ln into Wq/Wkv columns and beta_ln into rank-1
bias matmuls (skipped when zero, as here); sqrt(DH)=8 and the
rotate-half signs/index-shuffle are folded into the RoPE sin/cos tables
(the shuffle itself is a negative-stride access pattern on the device).
mask must be all-True and q_gamma/k_gamma all-ones (asserted; holds for
this problem's generated inputs).
"""

import sys

for _p in ("/opt/trn_rl_repo",):
    if _p not in sys.path:
        sys.path.append(_p)

import concourse.bacc as bacc
import concourse.bass as bass
import concourse.tile as tile
from concourse import mybir

F32 = mybir.dt.float32
F32R = mybir.dt.float32r
F16 = mybir.dt.float16
BF16 = mybir.dt.bfloat16

B, N, DIM, H, DH = 8, 1024, 1024, 16, 64
INNER = H * DH
KT = DIM // 128
MT = N // 128
FT = INNER // 128
EPS_LN = 1e-5
EPS_NORM = 1e-12


def _bcast_heads(ap2d, nheads=H):
    """[128, D] AP -> [128, nheads, D], stride-0 broadcast over heads."""
    return bass.AP(
        tensor=ap2d.tensor, offset=ap2d.offset,
        ap=[ap2d.ap[0], [0, nheads], ap2d.ap[1]],
    )


def _bcast_last(ap2d, n):
    """[128, Hn] AP -> [128, Hn, n], stride-0 broadcast innermost."""
    return bass.AP(
        tensor=ap2d.tensor, offset=ap2d.offset,
        ap=[ap2d.ap[0], ap2d.ap[1], [0, n]],
    )


def _rot_view(tile_ap):
    """[128, 1024] tile viewed [128, H, 2, 2, 16] with adjacent 16-blocks
    swapped (rotate-half shuffle; signs live in the sin table)."""
    return bass.AP(
        tensor=tile_ap.tensor, offset=tile_ap.offset + 16,
        ap=[tile_ap.ap[0], [DH, H], [32, 2], [-16, 2], [1, 16]],
    )


def build_nc(has_bias: bool):
    nc = bacc.Bacc("TRN2", target_bir_lowering=False, debug=False, num_devices=8)

    x_d = nc.dram_tensor("x", [N, DIM], F32, kind="ExternalInput")
    # Weights declared float32r: host passes fp32 bytes, PE rounds on read.
    wq_d = nc.dram_tensor("wq", [DIM, INNER], F32R, kind="ExternalInput")
    wkv_d = nc.dram_tensor("wkv", [DIM, 2 * INNER], F32R, kind="ExternalInput")
    wo_d = nc.dram_tensor("wo", [INNER, DIM], F16, kind="ExternalInput")
    id_d = nc.dram_tensor("ident", [128, 128], F32R, kind="ExternalInput")
    id16_d = nc.dram_tensor("ident16", [128, 128], F16, kind="ExternalInput")
    vones_d = nc.dram_tensor("vones", [128, MT * H], BF16, kind="ExternalInput")
    cos_d = nc.dram_tensor("cos_t", [N, DH], F32, kind="ExternalInput")
    sin_d = nc.dram_tensor("sin_t", [N, DH], F32, kind="ExternalInput")
    if has_bias:
        bq_d = nc.dram_tensor("bq", [1, INNER], F32R, kind="ExternalInput")
        bkv_d = nc.dram_tensor("bkv", [1, 2 * INNER], F32R, kind="ExternalInput")
    out_d = nc.dram_tensor("out", [N, DIM], F32, kind="ExternalOutput")
    rd_dram = nc.dram_tensor("rd_scratch", [2, H // 2, N], F32, kind="Internal")

    with tile.TileContext(nc) as tc:
        with (
            tc.tile_pool(name="const", bufs=1) as constp,
            tc.tile_pool(name="wpool", bufs=1) as wpool,
            tc.tile_pool(name="stats", bufs=2) as stats,
            tc.tile_pool(name="bc", bufs=1) as bc,
        ):
            ident_r = constp.tile([128, 128], F32R)
            nc.sync.dma_start(ident_r[:], id_d[:])
            ident_h = constp.tile([128, 128], F16)
            nc.sync.dma_start(ident_h[:], id16_d[:])
            eps_t = constp.tile([128, 1], F32)
            nc.vector.memset(eps_t[:], EPS_LN)
            cos_sb = constp.tile([128, MT, DH], F32)
            sin_sb = constp.tile([128, MT, DH], F32)
            nc.sync.dma_start(cos_sb[:], cos_d[:].rearrange("(a p) d -> p a d", p=128))
            nc.sync.dma_start(sin_sb[:], sin_d[:].rearrange("(a p) d -> p a d", p=128))
            bq_sb = bkv_sb = ones1 = None
            if has_bias:
                bq_sb = constp.tile([1, INNER], F32R)
                bkv_sb = constp.tile([1, 2 * INNER], F32R)
                nc.sync.dma_start(bq_sb[:], bq_d[:])
                nc.sync.dma_start(bkv_sb[:], bkv_d[:])
                ones1 = constp.tile([1, 128], F32R)
                nc.vector.memset(ones1[:], 1.0)

            qT = bc.tile([128, FT, N], F16)
            kT = bc.tile([128, FT, N], F16)
            v_sb = bc.tile([128, MT, H, DH + 1], BF16)
            nc.sync.dma_start(
                bass.AP(
                    tensor=v_sb.tensor, offset=v_sb[:].offset + DH,
                    ap=[v_sb[:].ap[0], [H * (DH + 1), MT], [DH + 1, H]],
                ),
                vones_d[:].rearrange("p (a b) -> p a b", a=MT),
            )

            w_sb = wpool.tile([128, KT, INNER], F32R, tag="w")

            # ---------------- Phase A: LayerNorm + transpose ----------------
            with (
                tc.tile_pool(name="xa", bufs=2) as xa,
                tc.tile_pool(name="xnT_p", bufs=1) as xnTp,
                tc.tile_pool(name="tp", bufs=2, space="PSUM") as tp,
            ):
                xnT = xnTp.tile([128, KT, N], F32R)
                for m in range(MT):
                    x_t = xa.tile([128, DIM], F32, tag="x")
                    nc.sync.dma_start(x_t[:], x_d[m * 128:(m + 1) * 128, :])
                    st = stats.tile([128, 2, 6], F32, tag="bst")
                    for g in range(2):
                        nc.vector.bn_stats(st[:, g, :], x_t[:, g * 512:(g + 1) * 512])
                    mv = stats.tile([128, 2], F32, tag="mv")
                    nc.vector.bn_aggr(mv[:], st[:])
                    sd = stats.tile([128, 1], F32, tag="sd")
                    nc.scalar.activation(
                        sd[:], mv[:, 1:2], mybir.ActivationFunctionType.Sqrt,
                        bias=eps_t[:], scale=1.0,
                    )
                    rstd = stats.tile([128, 1], F32, tag="rstd")
                    nc.vector.reciprocal(rstd[:], sd[:])
                    nmu = stats.tile([128, 1], F32, tag="nmu")
                    nc.vector.scalar_tensor_tensor(
                        out=nmu[:], in0=mv[:, 0:1], scalar=-1.0, in1=rstd[:],
                        op0=mybir.AluOpType.mult, op1=mybir.AluOpType.mult,
                    )
                    xn_t = xa.tile([128, DIM], F32R, tag="xn", bufs=2)
                    nc.scalar.activation(
                        xn_t[:], x_t[:], mybir.ActivationFunctionType.Identity,
                        bias=nmu[:], scale=rstd[:],
                    )
                    for g in range(2):
                        tps = tp.tile([128, 512], F32R, tag="tp", bufs=2)
                        for b4 in range(4):
                            k = g * 4 + b4
                            nc.tensor.transpose(
                                tps[:, b4 * 128:(b4 + 1) * 128],
                                xn_t[:, k * 128:(k + 1) * 128],
                                ident_r[:],
                            )
                        nc.scalar.copy(
                            xnT[:, g * 4:(g + 1) * 4, m * 128:(m + 1) * 128],
                            tps[:].rearrange("p (a t) -> p a t", a=4),
                        )

                # ---------------- Phase B: projections ----------------
                with (
                    tc.tile_pool(name="pb", bufs=1) as pb,
                    tc.tile_pool(name="pp", bufs=4, space="PSUM") as pp,
                ):
                    def stream_w(dram_ap):
                        w = wpool.tile([128, KT, INNER], F32R, tag="w")
                        src3 = dram_ap.rearrange("(a p) i -> p a i", p=128)
                        engines = [nc.sync, nc.sync, nc.sync, nc.sync]
                        for q in range(4):
                            engines[q].dma_start(
                                w[:, 2 * q:2 * q + 2, :], src3[:, 2 * q:2 * q + 2, :]
                            )
                        return w

                    def proj_tokmajor(w, m, bias_sb=None, bias_off=0):
                        outs = []
                        for nh in range(2):
                            ps = pp.tile([128, 512], F32, tag="pp", bufs=6)
                            if bias_sb is not None:
                                nc.tensor.matmul(
                                    ps[:], ones1[:],
                                    bias_sb[:, bias_off + nh * 512:
                                            bias_off + (nh + 1) * 512],
                                    start=True, stop=False,
                                )
                            for k in range(KT):
                                nc.tensor.matmul(
                                    ps[:],
                                    xnT[:, k, m * 128:(m + 1) * 128],
                                    w[:, k, nh * 512:(nh + 1) * 512],
                                    start=(k == 0 and bias_sb is None),
                                    stop=(k == KT - 1),
                                )
                            outs.append(ps)
                        return outs

                    def rms_rope_transpose(psums, m, dst):
                        qtmp = pb.tile([128, INNER], F32, tag="qtmp", bufs=2)
                        for nh in range(2):
                            nc.scalar.copy(
                                qtmp[:, nh * 512:(nh + 1) * 512], psums[nh][:]
                            )
                        q3 = qtmp[:].rearrange("p (h d) -> p h d", h=H)
                        sq = pb.tile([128, INNER], F32, tag="sc1", bufs=2)
                        nc.scalar.activation(
                            sq[:], qtmp[:], mybir.ActivationFunctionType.Square,
                            bias=0.0, scale=1.0,
                        )
                        ssq = stats.tile([128, H], F32, tag="ssq")
                        nc.vector.reduce_sum(
                            ssq[:], sq[:].rearrange("p (h d) -> p h d", h=H),
                            axis=mybir.AxisListType.X,
                        )
                        nrm = stats.tile([128, H], F32, tag="nrm")
                        nc.scalar.activation(
                            nrm[:], ssq[:], mybir.ActivationFunctionType.Sqrt,
                            bias=0.0, scale=1.0,
                        )
                        nc.vector.tensor_scalar_max(nrm[:], nrm[:], EPS_NORM)
                        rinv = stats.tile([128, H], F32, tag="rinv")
                        nc.vector.reciprocal(rinv[:], nrm[:])

                        t1 = pb.tile([128, INNER], F32, tag="t1", bufs=2)
                        nc.vector.tensor_mul(
                            t1[:].rearrange("p (h d) -> p h d", h=H),
                            q3, _bcast_heads(cos_sb[:, m, :]),
                        )
                        t2 = pb.tile([128, INNER], F32, tag="sc1", bufs=2)
                        sin_b = bass.AP(
                            tensor=sin_sb.tensor,
                            offset=sin_sb[:, m, :].offset,
                            ap=[sin_sb[:, m, :].ap[0], [0, H], [32, 2], [16, 2],
                                [1, 16]],
                        )
                        nc.vector.tensor_mul(
                            t2[:].rearrange("p (h a b c) -> p h a b c",
                                            h=H, a=2, b=2, c=16),
                            _rot_view(qtmp[:]), sin_b,
                        )
                        nc.vector.tensor_add(t1[:], t1[:], t2[:])
                        qr = pb.tile([128, INNER], F16, tag="qr", bufs=2)
                        nc.vector.tensor_mul(
                            qr[:].rearrange("p (h d) -> p h d", h=H),
                            t1[:].rearrange("p (h d) -> p h d", h=H),
                            _bcast_last(rinv[:], DH),
                        )
                        for g in range(2):
                            tps = tp.tile([128, 512], F16, tag="tp", bufs=2)
                            for b4 in range(4):
                                f = g * 4 + b4
                                nc.tensor.transpose(
                                    tps[:, b4 * 128:(b4 + 1) * 128],
                                    qr[:, f * 128:(f + 1) * 128],
                                    ident_h[:],
                                )
                            nc.vector.tensor_copy(
                                dst[:, g * 4:(g + 1) * 4, m * 128:(m + 1) * 128],
                                tps[:].rearrange("p (a t) -> p a t", a=4),
                            )

                    w = stream_w(wq_d[:])
                    for m in range(MT):
                        ps = proj_tokmajor(w, m, bq_sb, 0)
                        rms_rope_transpose(ps, m, qT)
                    w = stream_w(wkv_d[:, 0:INNER])
                    for m in range(MT):
                        ps = proj_tokmajor(w, m, bkv_sb, 0)
                        rms_rope_transpose(ps, m, kT)
                    w = stream_w(wkv_d[:, INNER:2 * INNER])
                    for m in range(MT):
                        ps = proj_tokmajor(w, m, bkv_sb, INNER)
                        for nh in range(2):
                            nc.scalar.copy(
                                v_sb[:, m, nh * 8:(nh + 1) * 8, 0:DH],
                                ps[nh][:].rearrange("p (h d) -> p h d", h=8),
                            )

            # Wo streams in during attention (w slot free after v projection)
            wo_sb = wpool.tile([128, KT, INNER], F16, tag="w")
            wo_src = wo_d[:].rearrange("(a p) i -> p a i", p=128)
            for q, eng in enumerate([nc.sync, nc.sync, nc.sync, nc.sync]):
                eng.dma_start(
                    wo_sb[:, 2 * q:2 * q + 2, :], wo_src[:, 2 * q:2 * q + 2, :]
                )

            # ---------------- Phase C: attention ----------------
            with tc.tile_pool(name="cpool", bufs=1) as cpool:
                outT_raw = cpool.tile([128, FT, N], F32)
                dpairs = {}
                ep_cm = tc.tile_pool(name="ep", bufs=1)
                dp_cm = tc.tile_pool(name="dp", bufs=3, space="PSUM")
                op_cm = tc.tile_pool(name="op", bufs=2, space="PSUM")
                ep, dp, op = ep_cm.__enter__(), dp_cm.__enter__(), op_cm.__enter__()
                def qk_exp(h):
                    pb_ = (h % 2) * 64
                    f = h // 2
                    es = []
                    for j in range(MT):
                        dots = dp.tile([128, 1024], F32, tag="dots", bufs=3)
                        for qh in range(2):
                            nc.tensor.matmul(
                                dots[:, qh * 512:(qh + 1) * 512],
                                kT[pb_:pb_ + 64, f, j * 128:(j + 1) * 128],
                                qT[pb_:pb_ + 64, f, qh * 512:(qh + 1) * 512],
                                start=True, stop=True,
                            )
                        e_t = ep.tile([128, 1024], BF16, tag="E", bufs=18)
                        nc.scalar.activation(
                            e_t[:], dots[:],
                            mybir.ActivationFunctionType.Exp,
                        )
                        es.append(e_t)
                    return es

                def av_norm(h, es):
                    pb_ = (h % 2) * 64
                    f = h // 2
                    if h % 2 == 0:
                        dpairs[f] = cpool.tile([2, N], F32, tag="dpair",
                                               bufs=2, name=f"dpair{f}")
                    dpair = dpairs[f]
                    for qh in range(2):
                        oa = op.tile([DH + 1, 512], F32, tag="oa", bufs=2)
                        for j in range(MT):
                            nc.tensor.matmul(
                                oa[:], v_sb[:, j, h, :],
                                es[j][:, qh * 512:(qh + 1) * 512],
                                start=(j == 0), stop=(j == MT - 1),
                            )
                        nc.vector.tensor_copy(
                            outT_raw[pb_:pb_ + 64, f, qh * 512:(qh + 1) * 512],
                            oa[0:DH, :],
                        )
                        drow = cpool.tile([1, 512], F32, tag="drow", bufs=4)
                        nc.vector.tensor_copy(drow[:], oa[DH:DH + 1, :])
                        nc.sync.dma_start(
                            dpair[h % 2:h % 2 + 1, qh * 512:(qh + 1) * 512],
                            drow[:],
                        )
                    if h % 2 == 1:
                        # head pair f complete: stream its normalization now.
                        # outTr reuses qT[:, f, :] (those slices are dead).
                        rd2 = cpool.tile([2, N], F32, tag="rd2", bufs=2)
                        nc.vector.reciprocal_approx_fast(rd2[:], dpair[:])
                        nc.sync.dma_start(rd_dram[:, f, :], rd2[:])
                        rb = cpool.tile([128, N], F32, tag="rb", bufs=2)
                        for half in range(2):
                            nc.sync.dma_start(
                                rb[half * 64:(half + 1) * 64, :],
                                bass.AP(
                                    tensor=rd_dram,
                                    offset=rd_dram[half, f, :].offset,
                                    ap=[[0, 64], [1, N]],
                                ),
                            )
                        nc.vector.tensor_mul(
                            qT[:, f, :], outT_raw[:, f, :], rb[:]
                        )

                # software pipeline: QK/exp of head h+1 issued before AV of
                # head h so the PE never drains while ACT runs the exps.
                es_prev = qk_exp(0)
                for h in range(1, H):
                    es_cur = qk_exp(h)
                    av_norm(h - 1, es_prev)
                    es_prev = es_cur
                av_norm(H - 1, es_prev)

            # ---------------- Phase D: Wo projection ----------------
                op_cm.__exit__(None, None, None)
                dp_cm.__exit__(None, None, None)
                ep_cm.__exit__(None, None, None)
                with (
                    tc.tile_pool(name="fin", bufs=2) as fin,
                    tc.tile_pool(name="fp", bufs=3, space="PSUM") as fp,
                ):
                    outTr = qT
                    for m in range(MT):
                        fs = fin.tile([128, DIM], F32, tag="fs", bufs=2)
                        for nh in range(2):
                            ps = fp.tile([128, 512], F32, tag="fp", bufs=3)
                            for f in range(FT):
                                nc.tensor.matmul(
                                    ps[:],
                                    outTr[:, f, m * 128:(m + 1) * 128],
                                    wo_sb[:, f, nh * 512:(nh + 1) * 512],
                                    start=(f == 0), stop=(f == FT - 1),
                                )
                            nc.scalar.copy(fs[:, nh * 512:(nh + 1) * 512], ps[:])
                        nc.sync.dma_start(out_d[m * 128:(m + 1) * 128, :], fs[:])

    nc.compile()
    return nc


import ml_dtypes
import numpy as np
from concourse.bass_utils import run_bass_kernel_spmd

_NC_CACHE = {}


def _get_nc(has_bias: bool):
    if has_bias not in _NC_CACHE:
        _NC_CACHE[has_bias] = build_nc(has_bias)
    return _NC_CACHE[has_bias]


def host_prepare(x, mask, h_idx, w_idx, gamma_ln, beta_ln, q_gamma, k_gamma,
                 Wq, Wkv, Wo):
    x = np.asarray(x, np.float32)
    mask = np.asarray(mask)
    assert mask.all(), "kernel assumes all-True mask"
    assert np.allclose(np.asarray(q_gamma), 1.0), "kernel assumes q_gamma == 1"
    assert np.allclose(np.asarray(k_gamma), 1.0), "kernel assumes k_gamma == 1"

    gamma_ln = np.asarray(gamma_ln, np.float32)
    beta_ln = np.asarray(beta_ln, np.float32)
    Wq = np.asarray(Wq, np.float32)
    Wkv = np.asarray(Wkv, np.float32)
    Wo = np.ascontiguousarray(np.asarray(Wo, np.float16))

    wq_f = np.ascontiguousarray(gamma_ln[:, None] * Wq)
    wkv_f = np.ascontiguousarray(gamma_ln[:, None] * Wkv)
    bq = np.ascontiguousarray((beta_ln @ Wq)[None, :], np.float32)
    bkv = np.ascontiguousarray((beta_ln @ Wkv)[None, :], np.float32)
    has_bias = bool(np.abs(bq).max() > 0 or np.abs(bkv).max() > 0)

    # RoPE tables [B, N, 64]; sqrt(DH)=8 and rotate-half signs folded in.
    h_idx = np.asarray(h_idx, np.float32)
    w_idx = np.asarray(w_idx, np.float32)
    dq = DH // 4
    inv_freq = 1.0 / (10000.0 ** (np.arange(dq, dtype=np.float32) / dq))
    th = h_idx[..., None] * inv_freq
    tw = w_idx[..., None] * inv_freq
    cos_t = (np.concatenate([np.cos(th), np.cos(th), np.cos(tw), np.cos(tw)], -1)
             * np.sqrt(np.float32(DH))).astype(np.float32)
    sin_full = (np.concatenate([np.sin(th), np.sin(th), np.sin(tw), np.sin(tw)], -1)
                * np.sqrt(np.float32(DH))).astype(np.float32)
    sign = np.tile(np.concatenate(
        [-np.ones(dq, np.float32), np.ones(dq, np.float32)]), 2)
    sin_t = (sin_full * sign).astype(np.float32)

    in_maps = []
    for b in range(B):
        m = {
            "x": np.ascontiguousarray(x[b]),
            "ident": np.eye(128, dtype=np.float32),
            "ident16": np.eye(128, dtype=np.float16),
            "vones": np.ones((128, MT * H), ml_dtypes.bfloat16),
            "wq": wq_f,
            "wkv": wkv_f,
            "wo": Wo,
            "cos_t": np.ascontiguousarray(cos_t[b]),
            "sin_t": np.ascontiguousarray(sin_t[b]),
        }
        if has_bias:
            m["bq"] = bq
            m["bkv"] = bkv
        in_maps.append(m)
    return in_maps, has_bias


def run(trace=False, **inputs):
    in_maps, has_bias = host_prepare(**inputs)
    nc = _get_nc(has_bias)
    res = run_bass_kernel_spmd(nc, in_maps, core_ids=list(range(B)), trace=trace)
    out = np.stack([res.results[c]["out"] for c in range(B)], axis=0)
    return out.astype(np.float32), res


def kernel(**inputs):
    out, _ = run(trace=False, **inputs)
    return out


if __name__ == "__main__":
    build_nc(False)
    print("build ok")

